# revision 30
# baseline (speedup 1.0000x reference)
"""Trainium2 Bass kernel for nn_AttentionEdgeDecoder.

Reference computation (per batch b):
  hn = h[b,:4096,:], hg = h[b,4096,:]
  q = hg @ W_q  (single query, 8 heads x 16 dims)
  k,v = hn @ W_kv ; attn = softmax(q.k/sqrt(16)) ; y = attn.v
  mh = y @ W_mhc ; y2[i] = <mh, hn[i]>             (4096 scalars)
  e[i,j] = y2[j]*W_lin[0,0] + y2[i]*W_lin[1,0]     (4096x4096 output)

Output is 4*4096^2*4B = 268MB -> HBM-write bound. Sharding: 8 cores =
4 batches x 2 row-halves; each core computes y2[b] redundantly (tiny) and
streams its (2048, 4096) block of e to DRAM at the per-core DMA-engine
limit (16 engines x ~27GB/s = ~430GB/s).

v3 layout, driven by perfetto/NTFF traces:
 - R = W0*y2[j] broadcast to 128 partitions lives ENTIRELY in PSUM
   (16KB/partition = all 8 banks, own pool phase) -> no PSUM->SBUF
   copies on the critical path; the e-tile adds read PSUM directly.
 - first output row-tile is added + DMA'd in 512-col pieces that chase
   the 8 R matmuls, so HBM writes start before R finishes.
 - remaining output in chunks of 1,2,4,4,4 row-tiles: partition p of a
   chunk holds TPC consecutive output rows -> one contiguous 16*TPC KB
   DMA descriptor (hrT is host-permuted so col[p] matches).
 - inputs stream in 4 chunks on both HWDGE rings; sT/exp/u are
   software-pipelined per 4-column-block group behind the DMA.
 - softmax denominator comes free from a ones-column appended to hnp.

TensorEngine formulation (out = lhsT.T @ rhs):
  q_col   = matmul(lhsT=W_q, rhs=hg_col)                  [128,1]  f32
  Qh      = headmask * q_col   (block-diag scatter)       [128,8]  f32
  Wqeff   = matmul(lhsT=WkT, rhs=Qh) = Wk @ Qh            [128,8]  ->bf16
  sT      = matmul(lhsT=hnT_chunk, rhs=Wqeff)             [4096,8] bf16 mm
  pT      = exp(0.25*sT)      (no max-subtract: |s/4| < 8)         ->bf16
  u'      = sum_chunks matmul(lhsT=pT_chunk, rhs=[hn|1])  [8,129]
  rs      = 1/u'[:,128] ; ubar = u'[:, :128] * rs -> bf16 [8,128]
  uT      = PE-transpose(ubar)  -> bf16                   [128,8]
  ymatT   = matmul(lhsT=Wv_bf, rhs=uT)                    [128,8]
  y_col   = reduce_h(ymatT * headmask) -> bf16            [128,1]
  mh_row  = matmul(lhsT=y_col, rhs=Wmhc_bf) -> bf16       [1,128]
  mh01    = matmul(lhsT=mh_row, rhs=Wl_row)               [128,2]
  mh0_rep = ones128 * mh01[:,0]  (DVE bcast)              [128,128] bf16
  col     = matmul(lhsT=hrT_tile, rhs=mh01[:,1]) = W1*y2[rows] [128,16]
  R       = matmul(lhsT=mh0_rep, rhs=hnT) in PSUM            [128,4096]
  e_tile  = tensor_scalar_add(R_psum, col[:,t]) -> DMA out
"""

from contextlib import ExitStack

import ml_dtypes
import numpy as np

import concourse.bass as bass
import concourse.mybir as mybir
from concourse import bacc, tile
from concourse.bass_utils import run_bass_kernel_spmd

BP = 4
N = 4096
HID = 128
HP1 = HID + 1           # hn chunk width incl. ones column
H = 8
D = 16
ROWS = N // 2           # 2048 rows per core
NT = ROWS // 128        # 16 row tiles per core
NJC = N // 128          # 32 node chunks
F32 = mybir.dt.float32
BF16 = mybir.dt.bfloat16

SCHED = (1, 1, 2, 4, 4, 4)   # row-tiles per output chunk
TPCMAX = max(SCHED)
NWARM = 6               # PE warm-up matmuls

# wsmall column layout (bf16): early weights gating the q chain
SWQ0 = 0               # W_q
SWKT0 = HID            # WkT = W_kv[:, :128].T
SMSK0 = 2 * HID        # head mask [128, 8]
SHG0 = 2 * HID + H     # hg column
SWL0 = SHG0 + 1        # W_lin row (partition 0)
WS_COLS = SWL0 + 2
# wrest (bf16): [Wv | W_mhc | identity]; wmisc (f32): head mask


def build_bass():
    nc = bacc.Bacc()

    # weights ship pre-cast to bf16: wsmall (68KB) gates the q chain and
    # lands ~0.8us after queue start; wrest/wmisc are needed only later
    wsmall_ext = nc.declare_dram_parameter("wsmall", [HID, WS_COLS], BF16, isOutput=False)
    wrest_ext = nc.declare_dram_parameter("wrest", [HID, 3 * HID], BF16, isOutput=False)
    hnT_ext = nc.declare_dram_parameter("hnT", [HID, N], BF16, isOutput=False)
    # hnp: hn pre-packed on host to [p, jc, c] = hn[jc*128+p, c], c=128 is ones
    hnp_ext = nc.declare_dram_parameter("hnp", [128, NJC * HP1], BF16, isOutput=False)
    hrT_ext = nc.declare_dram_parameter("hrT", [HID, ROWS], BF16, isOutput=False)
    out_ext = nc.declare_dram_parameter("out", [ROWS, N], F32, isOutput=True)

    with tile.TileContext(nc) as tc, ExitStack() as ctx:
        sb = ctx.enter_context(tc.tile_pool(name="sb", bufs=1))
        small = ctx.enter_context(tc.tile_pool(name="small", bufs=1))
        epool = ctx.enter_context(tc.tile_pool(name="epool", bufs=2))

        # constants first: the PE warm-up runs on a memset tile so it needs
        # no input DMA at all (f32-identity warmups used to stall sT by ~3.5us)
        ones128_bf = small.tile([128, HID], BF16)
        nc.vector.memset(ones128_bf[:], 1.0)

        # ---- input DMAs. Each HWDGE queue drains ~165 GB/s and small-elem
        # transfers trickle at the queue head, so: wsmall leads the sync
        # ring (gates the q chain), bulk hnT/wrest/hrT follow; hnp halves
        # get the scalar ring to themselves (they gate the u accumulation).
        wsmall_sb = sb.tile([HID, WS_COLS], BF16)
        nc.sync.dma_start(wsmall_sb[:], wsmall_ext[:, :])
        hnT_sb = sb.tile([HID, N], BF16)
        nc.sync.dma_start(hnT_sb[:, bass.ts(0, N // 2)], hnT_ext[:, bass.ts(0, N // 2)])
        hn_sb = sb.tile([128, NJC, HP1], BF16)
        hn_flat = hn_sb[:].rearrange("p a b -> p (a b)")
        nc.scalar.dma_start(hn_flat[:], hnp_ext[:, :])
        nc.scalar.dma_start(
            hnT_sb[:, bass.ts(1, N // 2)], hnT_ext[:, bass.ts(1, N // 2)]
        )
        # chain-time weights ride the gpsimd SWDGE queue (3rd dispatch lane)
        wrest_sb = sb.tile([HID, 3 * HID], BF16)
        nc.gpsimd.dma_start(wrest_sb[:], wrest_ext[:, :])
        hrT_sb = sb.tile([HID, ROWS], BF16)
        nc.gpsimd.dma_start(hrT_sb[:], hrT_ext[:, :])

        wq_bf = wsmall_sb[:, SWQ0:SWQ0 + HID]
        wkt_bf = wsmall_sb[:, SWKT0:SWKT0 + HID]
        maskb_ap = wsmall_sb[:, SMSK0:SMSK0 + H]
        hg_bf = wsmall_sb[:, SHG0:SHG0 + 1]
        wl_bf = wsmall_sb[:, SWL0:SWL0 + 2]
        wv_bf = wrest_sb[:, 0:HID]
        wmhc_bf = wrest_sb[:, HID:2 * HID]
        id_bf = wrest_sb[:, 2 * HID:3 * HID]
        # f32 mask for the ymm multiply, cast on-chip (a 32B-elem f32 DMA
        # would trickle 128 tiny descriptors through the queue head)
        mask_f32 = small.tile([HID, H], F32)
        nc.vector.tensor_copy(mask_f32[:], maskb_ap)
        mask_ap = mask_f32[:]

        col_sb = small.tile([128, NT], F32)
        mh0rep_sb = small.tile([HID, HID], BF16)
        mh1_bf = small.tile([HID, 1], BF16)

        # ================= phase A: attention prologue =================
        with tc.tile_pool(name="ps_pre", bufs=1, space="PSUM") as ps_pre:
            # PE warm-up: dependency-free matmuls at t~0 on the memset tile
            for w in range(NWARM):
                warm_ps = ps_pre.tile([128, HID], F32, tag="warm", bufs=2)
                nc.tensor.matmul(
                    warm_ps[:], ones128_bf[:], ones128_bf[:], start=True, stop=True
                )

            q_ps = ps_pre.tile([HID, 1], F32, tag="tmp", bufs=2, padded_shape=[128, HID])
            nc.tensor.matmul(q_ps[:], wq_bf, hg_bf, start=True, stop=True)
            q_sb = small.tile([HID, 1], F32)
            nc.vector.tensor_copy(q_sb[:], q_ps[:])

            # Qh block-diag scatter: Qh[e, h] = mask[e, h] * q[e]
            qh_bf = small.tile([HID, H], BF16)
            nc.vector.tensor_scalar_mul(qh_bf[:], maskb_ap, q_sb[:])

            # Wqeff = Wk @ Qh  (cast to bf16 on the PSUM->SBUF copy)
            wqeff_ps = ps_pre.tile([HID, H], F32, tag="tmp", bufs=2, padded_shape=[128, HID])
            nc.tensor.matmul(wqeff_ps[:], wkt_bf, qh_bf[:], start=True, stop=True)
            wqeff_sb = small.tile([HID, H], BF16)
            nc.vector.tensor_copy(wqeff_sb[:], wqeff_ps[:])

            # sT / exp / u software-pipelined in groups of 4 chunks
            sT_ps = ps_pre.tile([128, NJC, H], F32, tag="sT")
            pT_sb = small.tile([128, NJC, H], BF16)
            u_ps = ps_pre.tile([H, HP1], F32, tag="u")
            # u lags sT by TWO groups so the PE never stalls on the
            # scalar-engine exp round trip
            NGRP = NJC // 4
            for g in range(NGRP):
                for jc in range(g * 4, g * 4 + 4):
                    nc.tensor.matmul(
                        sT_ps[:, jc, :],
                        hnT_sb[:, bass.ts(jc, 128)],
                        wqeff_sb[:],
                        start=True,
                        stop=True,
                    )
                nc.scalar.activation(
                    pT_sb[:, g * 4:(g + 1) * 4, :],
                    sT_ps[:, g * 4:(g + 1) * 4, :],
                    mybir.ActivationFunctionType.Exp,
                    scale=0.25,
                )
                if g >= 2:
                    for jc in range((g - 2) * 4, (g - 2) * 4 + 4):
                        nc.tensor.matmul(
                            u_ps[:],
                            pT_sb[:, jc, :],
                            hn_sb[:, jc, :],
                            start=(jc == 0),
                            stop=False,
                        )
            for jc in range(NJC - 8, NJC):
                nc.tensor.matmul(
                    u_ps[:], pT_sb[:, jc, :], hn_sb[:, jc, :],
                    start=False, stop=(jc == NJC - 1),
                )

            # rs = 1/ssum directly from the ones-column of u'
            rs_sb = small.tile([H, 1], F32)
            nc.vector.reciprocal(rs_sb[:], u_ps[:, HID:HP1])
            ubar_bf = small.tile([H, HID], BF16)
            nc.vector.tensor_scalar_mul(ubar_bf[:], u_ps[:, 0:HID], rs_sb[:])

            uT_ps = ps_pre.tile([HID, H], BF16, tag="tmp", bufs=2, padded_shape=[128, HID])
            nc.tensor.transpose(uT_ps[:], ubar_bf[:], id_bf[0:H, 0:H])
            uT_bf = small.tile([HID, H], BF16)
            nc.vector.tensor_copy(uT_bf[:], uT_ps[:])

            # ymatT = Wv.T @ uT  -> [e, h]
            ymatT_ps = ps_pre.tile([HID, H], F32, tag="tmp", bufs=2, padded_shape=[128, HID])
            nc.tensor.matmul(ymatT_ps[:], wv_bf, uT_bf[:], start=True, stop=True)
            # y_col[e] = ymatT[e, head(e)] = sum_h ymatT[e, h] * mask[e, h]
            ymm_sb = small.tile([HID, H], F32)
            y_bf = small.tile([HID, 1], BF16)
            nc.vector.tensor_mul(ymm_sb[:], ymatT_ps[:], mask_ap)
            with nc.allow_low_precision(reason="y is O(1); bf16 out is fine"):
                nc.vector.tensor_reduce(
                    y_bf[:], ymm_sb[:], axis=mybir.AxisListType.X, op=mybir.AluOpType.add
                )

            # mh_row = y.T @ W_mhc
            mh_ps = ps_pre.tile([1, HID], F32, tag="tmp", bufs=2, padded_shape=[128, HID])
            nc.tensor.matmul(mh_ps[:], y_bf[:], wmhc_bf, start=True, stop=True)
            mh_bf = small.tile([1, HID], BF16)
            nc.vector.tensor_copy(mh_bf[:], mh_ps[:])

            # mh01[c, :] = [W0*mh[c], W1*mh[c]]  (K=1 transpose-ish matmul)
            mh01_ps = ps_pre.tile([HID, 2], F32, tag="tmp", bufs=2, padded_shape=[128, HID])
            nc.tensor.matmul(mh01_ps[:], mh_bf[:], wl_bf[0:1, 0:2], start=True, stop=True)
            nc.scalar.copy(mh1_bf[:], mh01_ps[:, 1:2])
            # mh0_rep[c, p] = W0*mh[c]  (DVE per-partition broadcast straight
            # from PSUM)
            nc.vector.tensor_scalar_mul(mh0rep_sb[:], ones128_bf[:], mh01_ps[:, 0:1])

            # col[p, t] = W1*y2[perm row] (host-permuted hrT matches SCHED)
            col_ps = ps_pre.tile([128, NT], F32, tag="col")
            for t in range(NT):
                nc.tensor.matmul(
                    col_ps[:, t:t + 1],
                    hrT_sb[:, bass.ts(t, 128)],
                    mh1_bf[:],
                    start=True,
                    stop=True,
                )
            nc.vector.tensor_copy(col_sb[:], col_ps[:])

        # ================= phase B: R in PSUM + epilogue =================
        # DVE reads PSUM at ~half its SBUF rate, so only the first two
        # row-tiles add directly from PSUM (512-col pieces chasing the R
        # matmuls -> HBM writes start before R completes). Meanwhile the
        # scalar engine copies R into SBUF; all later tiles add from SBUF
        # at full DVE rate to stay ahead of the DMA drain.
        r_sb = sb.tile([128, N], F32)
        with tc.tile_pool(name="ps_R", bufs=1, space="PSUM") as ps_R:
            r_ps = ps_R.tile([128, N], F32)
            for k in range(8):
                nc.tensor.matmul(
                    r_ps[:, bass.ts(k, 512)], mh0rep_sb[:], hnT_sb[:, bass.ts(k, 512)],
                    start=True, stop=True,
                )

            # chunk 0: 512-col adds chase R in PSUM, then ONE full-width DMA
            # (16KB descriptors stream at full engine rate; 2KB pieces don't)
            etile0 = epool.tile([128, TPCMAX, N], F32, tag="e")
            for k in range(8):
                nc.vector.tensor_scalar_add(
                    etile0[:, 0, bass.ts(k, 512)], r_ps[:, bass.ts(k, 512)],
                    col_sb[:, 0:1],
                )
                nc.scalar.copy(r_sb[:, bass.ts(k, 512)], r_ps[:, bass.ts(k, 512)])
            nc.sync.dma_start(out_ext[0:128, :], etile0[:, 0, :])

        # remaining chunks per SCHED, added from SBUF
        r0 = 128
        cidx = 1
        for tpc in SCHED[1:]:
            etile = epool.tile([128, TPCMAX, N], F32, tag="e")
            for s in range(tpc):
                colv = col_sb[:, cidx + s:cidx + s + 1]
                nc.vector.tensor_scalar_add(etile[:, s, :], r_sb[:], colv)
            dst = out_ext[r0:r0 + tpc * 128, :].rearrange(
                "(p s) j -> p s j", p=128, s=tpc
            )
            nc.sync.dma_start(dst, etile[:, 0:tpc, :])
            r0 += tpc * 128
            cidx += tpc

    nc.finalize()
    return nc


_CACHED = {}


def _get_nc():
    if "nc" not in _CACHED:
        _CACHED["nc"] = build_bass()
    return _CACHED["nc"]


def _make_mask():
    mask = np.zeros((HID, H), dtype=np.float32)
    for hh in range(H):
        mask[hh * D:(hh + 1) * D, hh] = 1.0
    return mask


def _make_wsmall(W_q, W_kv, W_lin, mask):
    ws = np.zeros((HID, WS_COLS), dtype=np.float32)
    ws[:, SWQ0:SWQ0 + HID] = W_q
    ws[:, SWKT0:SWKT0 + HID] = W_kv[:, :HID].T
    ws[:, SMSK0:SMSK0 + H] = mask
    ws[0, SWL0] = W_lin[0, 0]
    ws[0, SWL0 + 1] = W_lin[1, 0]
    return ws  # hg column filled per core, then cast


def _make_wrest(W_kv, W_mhc):
    wr = np.empty((HID, 3 * HID), dtype=np.float32)
    wr[:, 0:HID] = W_kv[:, HID:]
    wr[:, HID:2 * HID] = W_mhc
    wr[:, 2 * HID:3 * HID] = np.eye(HID, dtype=np.float32)
    return wr.astype(ml_dtypes.bfloat16)


def _row_perm():
    # hrT column cidx*128+p  <->  local output row r0 + tpc*p + s
    perm = np.empty(ROWS, dtype=np.int64)
    cidx = 0
    r0 = 0
    for tpc in SCHED:
        for s in range(tpc):
            perm[cidx * 128:(cidx + 1) * 128] = r0 + tpc * np.arange(128) + s
            cidx += 1
        r0 += tpc * 128
    return perm


_PERM = _row_perm()


def kernel(h, W_q, W_kv, W_mhc, W_lin, _trace=False):
    h = np.ascontiguousarray(np.asarray(h, dtype=np.float32))
    W_q = np.asarray(W_q, dtype=np.float32)
    W_kv = np.asarray(W_kv, dtype=np.float32)
    W_mhc = np.asarray(W_mhc, dtype=np.float32)
    W_lin = np.asarray(W_lin, dtype=np.float32)

    nc = _get_nc()
    mask = _make_mask()
    ws0 = _make_wsmall(W_q, W_kv, W_lin, mask)
    wrest = _make_wrest(W_kv, W_mhc)

    in_maps = []
    for core in range(8):
        b, half = core // 2, core % 2
        hn = h[b, :N, :]
        ws = ws0.copy()
        ws[:, SHG0] = h[b, N, :]
        hnb = hn.astype(ml_dtypes.bfloat16)
        # hnp[p, jc*129 + c] = hn[jc*128 + p, c]; column 128 = 1.0
        hnp = np.ones((128, NJC, HP1), dtype=ml_dtypes.bfloat16)
        hnp[:, :, :HID] = hnb.reshape(NJC, 128, HID).transpose(1, 0, 2)
        hnp = np.ascontiguousarray(hnp.reshape(128, NJC * HP1))
        hr = hnb[half * ROWS:(half + 1) * ROWS, :][_PERM]
        in_maps.append(
            {
                "wsmall": ws.astype(ml_dtypes.bfloat16),
                "wrest": wrest,
                "hnT": np.ascontiguousarray(hnb.T),
                "hnp": hnp,
                "hrT": np.ascontiguousarray(hr.T),
            }
        )

    import time as _time

    kw = {}
    if _trace:
        import os

        kw = {"tmpdir": "/tmp/ktrace_" + str(os.getpid())}
        os.makedirs(kw["tmpdir"], exist_ok=True)
        print("[kernel] trace dir:", kw["tmpdir"], flush=True)
    _t = _time.time()
    print("[kernel] launching run_bass_kernel_spmd", flush=True)
    res = run_bass_kernel_spmd(nc, in_maps, core_ids=list(range(8)), trace=_trace, **kw)
    print(f"[kernel] run_bass_kernel_spmd done in {_time.time()-_t:.1f}s", flush=True)

    out = np.empty((BP, N * N, 1), dtype=np.float32)
    for core in range(8):
        b, half = core // 2, core % 2
        blk = res.results[core]["out"]  # (2048, 4096)
        out[b, half * ROWS * N:(half + 1) * ROWS * N, 0] = blk.ravel()
    if _trace:
        return out, res
    return out


# revision 31
# speedup vs baseline: 1.1977x; 1.1977x over previous
"""Trainium2 Bass kernel for nn_AttentionEdgeDecoder.

Reference computation (per batch b):
  hn = h[b,:4096,:], hg = h[b,4096,:]
  q = hg @ W_q  (single query, 8 heads x 16 dims)
  k,v = hn @ W_kv ; attn = softmax(q.k/sqrt(16)) ; y = attn.v
  mh = y @ W_mhc ; y2[i] = <mh, hn[i]>             (4096 scalars)
  e[i,j] = y2[j]*W_lin[0,0] + y2[i]*W_lin[1,0]     (4096x4096 output)

Output is 4*4096^2*4B = 268MB -> HBM-write bound. Sharding: 8 cores =
4 batches x 2 row-halves; each core computes y2[b] redundantly (tiny) and
streams its (2048, 4096) block of e to DRAM at the per-core DMA-engine
limit (16 engines x ~27GB/s = ~430GB/s).

v3 layout, driven by perfetto/NTFF traces:
 - R = W0*y2[j] broadcast to 128 partitions lives ENTIRELY in PSUM
   (16KB/partition = all 8 banks, own pool phase) -> no PSUM->SBUF
   copies on the critical path; the e-tile adds read PSUM directly.
 - first output row-tile is added + DMA'd in 512-col pieces that chase
   the 8 R matmuls, so HBM writes start before R finishes.
 - remaining output in chunks of 1,2,4,4,4 row-tiles: partition p of a
   chunk holds TPC consecutive output rows -> one contiguous 16*TPC KB
   DMA descriptor (hrT is host-permuted so col[p] matches).
 - inputs stream in 4 chunks on both HWDGE rings; sT/exp/u are
   software-pipelined per 4-column-block group behind the DMA.
 - softmax denominator comes free from a ones-column appended to hnp.

TensorEngine formulation (out = lhsT.T @ rhs):
  q_col   = matmul(lhsT=W_q, rhs=hg_col)                  [128,1]  f32
  Qh      = headmask * q_col   (block-diag scatter)       [128,8]  f32
  Wqeff   = matmul(lhsT=WkT, rhs=Qh) = Wk @ Qh            [128,8]  ->bf16
  sT      = matmul(lhsT=hnT_chunk, rhs=Wqeff)             [4096,8] bf16 mm
  pT      = exp(0.25*sT)      (no max-subtract: |s/4| < 8)         ->bf16
  u'      = sum_chunks matmul(lhsT=pT_chunk, rhs=[hn|1])  [8,129]
  rs      = 1/u'[:,128] ; ubar = u'[:, :128] * rs -> bf16 [8,128]
  uT      = PE-transpose(ubar)  -> bf16                   [128,8]
  ymatT   = matmul(lhsT=Wv_bf, rhs=uT)                    [128,8]
  y_col   = reduce_h(ymatT * headmask) -> bf16            [128,1]
  mh_row  = matmul(lhsT=y_col, rhs=Wmhc_bf) -> bf16       [1,128]
  mh01    = matmul(lhsT=mh_row, rhs=Wl_row)               [128,2]
  mh0_rep = ones128 * mh01[:,0]  (DVE bcast)              [128,128] bf16
  col     = matmul(lhsT=hrT_tile, rhs=mh01[:,1]) = W1*y2[rows] [128,16]
  R       = matmul(lhsT=mh0_rep, rhs=hnT) in PSUM            [128,4096]
  e_tile  = tensor_scalar_add(R_psum, col[:,t]) -> DMA out
"""

from contextlib import ExitStack

import ml_dtypes
import numpy as np

import concourse.bass as bass
import concourse.mybir as mybir
from concourse import bacc, tile
from concourse.bass_utils import run_bass_kernel_spmd

BP = 4
N = 4096
HID = 128
HP1 = HID + 1           # hn chunk width incl. ones column
H = 8
D = 16
ROWS = N // 2           # 2048 rows per core
NT = ROWS // 128        # 16 row tiles per core
NJC = N // 128          # 32 node chunks
F32 = mybir.dt.float32
BF16 = mybir.dt.bfloat16

SCHED = (1, 1, 2, 4, 4, 4)   # row-tiles per output chunk
TPCMAX = max(SCHED)
NWARM = 6               # PE warm-up matmuls

# wsmall column layout (bf16): early weights gating the q chain
SWQ0 = 0               # W_q
SWKT0 = HID            # WkT = W_kv[:, :128].T
SMSK0 = 2 * HID        # head mask [128, 8]
SHG0 = 2 * HID + H     # hg column
SWL0 = SHG0 + 1        # W_lin row (partition 0)
WS_COLS = SWL0 + 2
# wrest (bf16): [Wv | W_mhc | identity]; wmisc (f32): head mask


def build_bass():
    nc = bacc.Bacc()

    # weights ship pre-cast to bf16: wsmall (68KB) gates the q chain and
    # lands ~0.8us after queue start; wrest/wmisc are needed only later
    wsmall_ext = nc.declare_dram_parameter("wsmall", [HID, WS_COLS], BF16, isOutput=False)
    wrest_ext = nc.declare_dram_parameter("wrest", [HID, 3 * HID], BF16, isOutput=False)
    hnT_ext = nc.declare_dram_parameter("hnT", [HID, N], BF16, isOutput=False)
    # hnp: hn pre-packed on host to [p, jc, c] = hn[jc*128+p, c], c=128 is ones
    hnp_ext = nc.declare_dram_parameter("hnp", [128, NJC * HP1], BF16, isOutput=False)
    hrT_ext = nc.declare_dram_parameter("hrT", [HID, ROWS], BF16, isOutput=False)
    out_ext = nc.declare_dram_parameter("out", [ROWS, N], F32, isOutput=True)

    with tile.TileContext(nc) as tc, ExitStack() as ctx:
        sb = ctx.enter_context(tc.tile_pool(name="sb", bufs=1))
        small = ctx.enter_context(tc.tile_pool(name="small", bufs=1))
        epool = ctx.enter_context(tc.tile_pool(name="epool", bufs=2))

        # constants first: the PE warm-up runs on a memset tile so it needs
        # no input DMA at all (f32-identity warmups used to stall sT by ~3.5us)
        ones128_bf = small.tile([128, HID], BF16)
        nc.vector.memset(ones128_bf[:], 1.0)

        # ---- input DMAs. Each HWDGE queue drains ~165 GB/s and small-elem
        # transfers trickle at the queue head, so: wsmall leads the sync
        # ring (gates the q chain), bulk hnT/wrest/hrT follow; hnp halves
        # get the scalar ring to themselves (they gate the u accumulation).
        wsmall_sb = sb.tile([HID, WS_COLS], BF16)
        nc.sync.dma_start(wsmall_sb[:], wsmall_ext[:, :])
        hnT_sb = sb.tile([HID, N], BF16)
        for k in range(2):
            nc.sync.dma_start(
                hnT_sb[:, bass.ts(k, N // 2)], hnT_ext[:, bass.ts(k, N // 2)]
            )
        hn_sb = sb.tile([128, NJC, HP1], BF16)
        hn_flat = hn_sb[:].rearrange("p a b -> p (a b)")
        for k in range(2):
            nc.scalar.dma_start(
                hn_flat[:, bass.ts(k, NJC * HP1 // 2)],
                hnp_ext[:, bass.ts(k, NJC * HP1 // 2)],
            )
        # chain-time weights ride the gpsimd SWDGE queue (3rd dispatch lane)
        wrest_sb = sb.tile([HID, 3 * HID], BF16)
        nc.gpsimd.dma_start(wrest_sb[:], wrest_ext[:, :])
        hrT_sb = sb.tile([HID, ROWS], BF16)
        nc.gpsimd.dma_start(hrT_sb[:], hrT_ext[:, :])

        wq_bf = wsmall_sb[:, SWQ0:SWQ0 + HID]
        wkt_bf = wsmall_sb[:, SWKT0:SWKT0 + HID]
        maskb_ap = wsmall_sb[:, SMSK0:SMSK0 + H]
        hg_bf = wsmall_sb[:, SHG0:SHG0 + 1]
        wl_bf = wsmall_sb[:, SWL0:SWL0 + 2]
        wv_bf = wrest_sb[:, 0:HID]
        wmhc_bf = wrest_sb[:, HID:2 * HID]
        id_bf = wrest_sb[:, 2 * HID:3 * HID]
        # f32 mask for the ymm multiply, cast on-chip (a 32B-elem f32 DMA
        # would trickle 128 tiny descriptors through the queue head)
        mask_f32 = small.tile([HID, H], F32)
        nc.vector.tensor_copy(mask_f32[:], maskb_ap)
        mask_ap = mask_f32[:]

        col_sb = small.tile([128, NT], F32)
        mh0rep_sb = small.tile([HID, HID], BF16)
        mh1_bf = small.tile([HID, 1], BF16)

        # ================= phase A: attention prologue =================
        with tc.tile_pool(name="ps_pre", bufs=1, space="PSUM") as ps_pre:
            # PE warm-up: dependency-free matmuls at t~0 on the memset tile
            for w in range(NWARM):
                warm_ps = ps_pre.tile([128, HID], F32, tag="warm", bufs=2)
                nc.tensor.matmul(
                    warm_ps[:], ones128_bf[:], ones128_bf[:], start=True, stop=True
                )

            q_ps = ps_pre.tile([HID, 1], F32, tag="tmp", bufs=2, padded_shape=[128, HID])
            nc.tensor.matmul(q_ps[:], wq_bf, hg_bf, start=True, stop=True)
            q_sb = small.tile([HID, 1], F32)
            nc.vector.tensor_copy(q_sb[:], q_ps[:])

            # Qh block-diag scatter: Qh[e, h] = mask[e, h] * q[e]
            qh_bf = small.tile([HID, H], BF16)
            nc.vector.tensor_scalar_mul(qh_bf[:], maskb_ap, q_sb[:])

            # Wqeff = Wk @ Qh  (cast to bf16 on the PSUM->SBUF copy)
            wqeff_ps = ps_pre.tile([HID, H], F32, tag="tmp", bufs=2, padded_shape=[128, HID])
            nc.tensor.matmul(wqeff_ps[:], wkt_bf, qh_bf[:], start=True, stop=True)
            wqeff_sb = small.tile([HID, H], BF16)
            nc.vector.tensor_copy(wqeff_sb[:], wqeff_ps[:])

            # sT / exp / u software-pipelined in groups of 4 chunks
            sT_ps = ps_pre.tile([128, NJC, H], F32, tag="sT")
            pT_sb = small.tile([128, NJC, H], BF16)
            u_ps = ps_pre.tile([H, HP1], F32, tag="u")
            # u lags sT by TWO groups so the PE never stalls on the
            # scalar-engine exp round trip
            NGRP = NJC // 4
            for g in range(NGRP):
                for jc in range(g * 4, g * 4 + 4):
                    nc.tensor.matmul(
                        sT_ps[:, jc, :],
                        hnT_sb[:, bass.ts(jc, 128)],
                        wqeff_sb[:],
                        start=True,
                        stop=True,
                    )
                nc.scalar.activation(
                    pT_sb[:, g * 4:(g + 1) * 4, :],
                    sT_ps[:, g * 4:(g + 1) * 4, :],
                    mybir.ActivationFunctionType.Exp,
                    scale=0.25,
                )
                if g >= 2:
                    for jc in range((g - 2) * 4, (g - 2) * 4 + 4):
                        nc.tensor.matmul(
                            u_ps[:],
                            pT_sb[:, jc, :],
                            hn_sb[:, jc, :],
                            start=(jc == 0),
                            stop=False,
                        )
            for jc in range(NJC - 8, NJC):
                nc.tensor.matmul(
                    u_ps[:], pT_sb[:, jc, :], hn_sb[:, jc, :],
                    start=False, stop=(jc == NJC - 1),
                )

            # rs = 1/ssum directly from the ones-column of u'
            rs_sb = small.tile([H, 1], F32)
            nc.vector.reciprocal(rs_sb[:], u_ps[:, HID:HP1])
            ubar_bf = small.tile([H, HID], BF16)
            nc.vector.tensor_scalar_mul(ubar_bf[:], u_ps[:, 0:HID], rs_sb[:])

            uT_ps = ps_pre.tile([HID, H], BF16, tag="tmp", bufs=2, padded_shape=[128, HID])
            nc.tensor.transpose(uT_ps[:], ubar_bf[:], id_bf[0:H, 0:H])
            uT_bf = small.tile([HID, H], BF16)
            nc.vector.tensor_copy(uT_bf[:], uT_ps[:])

            # ymatT = Wv.T @ uT  -> [e, h]
            ymatT_ps = ps_pre.tile([HID, H], F32, tag="tmp", bufs=2, padded_shape=[128, HID])
            nc.tensor.matmul(ymatT_ps[:], wv_bf, uT_bf[:], start=True, stop=True)
            # y_col[e] = ymatT[e, head(e)] = sum_h ymatT[e, h] * mask[e, h]
            ymm_sb = small.tile([HID, H], F32)
            y_bf = small.tile([HID, 1], BF16)
            nc.vector.tensor_mul(ymm_sb[:], ymatT_ps[:], mask_ap)
            with nc.allow_low_precision(reason="y is O(1); bf16 out is fine"):
                nc.vector.tensor_reduce(
                    y_bf[:], ymm_sb[:], axis=mybir.AxisListType.X, op=mybir.AluOpType.add
                )

            # mh_row = y.T @ W_mhc
            mh_ps = ps_pre.tile([1, HID], F32, tag="tmp", bufs=2, padded_shape=[128, HID])
            nc.tensor.matmul(mh_ps[:], y_bf[:], wmhc_bf, start=True, stop=True)
            mh_bf = small.tile([1, HID], BF16)
            nc.vector.tensor_copy(mh_bf[:], mh_ps[:])

            # mh01[c, :] = [W0*mh[c], W1*mh[c]]  (K=1 transpose-ish matmul)
            mh01_ps = ps_pre.tile([HID, 2], F32, tag="tmp", bufs=2, padded_shape=[128, HID])
            nc.tensor.matmul(mh01_ps[:], mh_bf[:], wl_bf[0:1, 0:2], start=True, stop=True)
            nc.scalar.copy(mh1_bf[:], mh01_ps[:, 1:2])
            # mh0_rep[c, p] = W0*mh[c]  (DVE per-partition broadcast straight
            # from PSUM)
            nc.vector.tensor_scalar_mul(mh0rep_sb[:], ones128_bf[:], mh01_ps[:, 0:1])

            # col[p, t] = W1*y2[perm row] (host-permuted hrT matches SCHED)
            col_ps = ps_pre.tile([128, NT], F32, tag="col")
            for t in range(NT):
                nc.tensor.matmul(
                    col_ps[:, t:t + 1],
                    hrT_sb[:, bass.ts(t, 128)],
                    mh1_bf[:],
                    start=True,
                    stop=True,
                )
            nc.vector.tensor_copy(col_sb[:], col_ps[:])

        # ================= phase B: R in PSUM + epilogue =================
        # DVE reads PSUM at ~half its SBUF rate, so only the first two
        # row-tiles add directly from PSUM (512-col pieces chasing the R
        # matmuls -> HBM writes start before R completes). Meanwhile the
        # scalar engine copies R into SBUF; all later tiles add from SBUF
        # at full DVE rate to stay ahead of the DMA drain.
        r_sb = sb.tile([128, N], F32)
        with tc.tile_pool(name="ps_R", bufs=1, space="PSUM") as ps_R:
            r_ps = ps_R.tile([128, N], F32)
            for k in range(8):
                nc.tensor.matmul(
                    r_ps[:, bass.ts(k, 512)], mh0rep_sb[:], hnT_sb[:, bass.ts(k, 512)],
                    start=True, stop=True,
                )

            # chunk 0: 512-col adds chase R in PSUM, then ONE full-width DMA
            # (16KB descriptors stream at full engine rate; 2KB pieces don't)
            etile0 = epool.tile([128, TPCMAX, N], F32, tag="e")
            for k in range(8):
                nc.vector.tensor_scalar_add(
                    etile0[:, 0, bass.ts(k, 512)], r_ps[:, bass.ts(k, 512)],
                    col_sb[:, 0:1],
                )
                nc.scalar.copy(r_sb[:, bass.ts(k, 512)], r_ps[:, bass.ts(k, 512)])
            nc.sync.dma_start(out_ext[0:128, :], etile0[:, 0, :])

        # remaining chunks per SCHED, added from SBUF
        r0 = 128
        cidx = 1
        for tpc in SCHED[1:]:
            etile = epool.tile([128, TPCMAX, N], F32, tag="e")
            for s in range(tpc):
                colv = col_sb[:, cidx + s:cidx + s + 1]
                nc.vector.tensor_scalar_add(etile[:, s, :], r_sb[:], colv)
            dst = out_ext[r0:r0 + tpc * 128, :].rearrange(
                "(p s) j -> p s j", p=128, s=tpc
            )
            nc.sync.dma_start(dst, etile[:, 0:tpc, :])
            r0 += tpc * 128
            cidx += tpc

    nc.finalize()
    return nc


_CACHED = {}


def _get_nc():
    if "nc" not in _CACHED:
        _CACHED["nc"] = build_bass()
    return _CACHED["nc"]


def _make_mask():
    mask = np.zeros((HID, H), dtype=np.float32)
    for hh in range(H):
        mask[hh * D:(hh + 1) * D, hh] = 1.0
    return mask


def _make_wsmall(W_q, W_kv, W_lin, mask):
    ws = np.zeros((HID, WS_COLS), dtype=np.float32)
    ws[:, SWQ0:SWQ0 + HID] = W_q
    ws[:, SWKT0:SWKT0 + HID] = W_kv[:, :HID].T
    ws[:, SMSK0:SMSK0 + H] = mask
    ws[0, SWL0] = W_lin[0, 0]
    ws[0, SWL0 + 1] = W_lin[1, 0]
    return ws  # hg column filled per core, then cast


def _make_wrest(W_kv, W_mhc):
    wr = np.empty((HID, 3 * HID), dtype=np.float32)
    wr[:, 0:HID] = W_kv[:, HID:]
    wr[:, HID:2 * HID] = W_mhc
    wr[:, 2 * HID:3 * HID] = np.eye(HID, dtype=np.float32)
    return wr.astype(ml_dtypes.bfloat16)


def _row_perm():
    # hrT column cidx*128+p  <->  local output row r0 + tpc*p + s
    perm = np.empty(ROWS, dtype=np.int64)
    cidx = 0
    r0 = 0
    for tpc in SCHED:
        for s in range(tpc):
            perm[cidx * 128:(cidx + 1) * 128] = r0 + tpc * np.arange(128) + s
            cidx += 1
        r0 += tpc * 128
    return perm


_PERM = _row_perm()


def kernel(h, W_q, W_kv, W_mhc, W_lin, _trace=False):
    h = np.ascontiguousarray(np.asarray(h, dtype=np.float32))
    W_q = np.asarray(W_q, dtype=np.float32)
    W_kv = np.asarray(W_kv, dtype=np.float32)
    W_mhc = np.asarray(W_mhc, dtype=np.float32)
    W_lin = np.asarray(W_lin, dtype=np.float32)

    nc = _get_nc()
    mask = _make_mask()
    ws0 = _make_wsmall(W_q, W_kv, W_lin, mask)
    wrest = _make_wrest(W_kv, W_mhc)

    in_maps = []
    for core in range(8):
        b, half = core // 2, core % 2
        hn = h[b, :N, :]
        ws = ws0.copy()
        ws[:, SHG0] = h[b, N, :]
        hnb = hn.astype(ml_dtypes.bfloat16)
        # hnp[p, jc*129 + c] = hn[jc*128 + p, c]; column 128 = 1.0
        hnp = np.ones((128, NJC, HP1), dtype=ml_dtypes.bfloat16)
        hnp[:, :, :HID] = hnb.reshape(NJC, 128, HID).transpose(1, 0, 2)
        hnp = np.ascontiguousarray(hnp.reshape(128, NJC * HP1))
        hr = hnb[half * ROWS:(half + 1) * ROWS, :][_PERM]
        in_maps.append(
            {
                "wsmall": ws.astype(ml_dtypes.bfloat16),
                "wrest": wrest,
                "hnT": np.ascontiguousarray(hnb.T),
                "hnp": hnp,
                "hrT": np.ascontiguousarray(hr.T),
            }
        )

    import time as _time

    kw = {}
    if _trace:
        import os

        kw = {"tmpdir": "/tmp/ktrace_" + str(os.getpid())}
        os.makedirs(kw["tmpdir"], exist_ok=True)
        print("[kernel] trace dir:", kw["tmpdir"], flush=True)
    _t = _time.time()
    print("[kernel] launching run_bass_kernel_spmd", flush=True)
    res = run_bass_kernel_spmd(nc, in_maps, core_ids=list(range(8)), trace=_trace, **kw)
    print(f"[kernel] run_bass_kernel_spmd done in {_time.time()-_t:.1f}s", flush=True)

    out = np.empty((BP, N * N, 1), dtype=np.float32)
    for core in range(8):
        b, half = core // 2, core % 2
        blk = res.results[core]["out"]  # (2048, 4096)
        out[b, half * ROWS * N:(half + 1) * ROWS * N, 0] = blk.ravel()
    if _trace:
        return out, res
    return out
